# revision 10
# baseline (speedup 1.0000x reference)
"""AuthPct metric kernel for 8 Trainium2 NeuronCores.

Sharding: real_stats rows are sharded across the 8 cores (1536 each).
For column features f_j each core computes PSUM tiles of

    Y[j, i] = 2*f_j.r_i - |r_i|^2 - |f_j|^2  =  -dist^2(f_j, r_i)

via bf16 PE matmuls: two K=128 feature chunks plus one augmented K=128
matmul (rows 0..3 of its operands carry the exact hi/lo bf16 splits of
-|r_i|^2 and -|f_j|^2; remaining rows are zero).  For the gen side the
aug lhsT has only the |r_i|^2 rows, so gen tiles hold X = 2G - |r_i|^2.

gen (96 j-tiles/core, all gen columns vs core rows): ScalarE copies
PSUM->SBUF wide [128,1536]; DVE `max` top-8 (d1 values) + `max_index`
(argmin payload for d2).

real: the distance matrix is symmetric, so each unordered shard pair is
computed once.  With host-rotated real columns every core runs the SAME
program on j-tiles covering shards c..c+4 (60 j-tiles): DVE `max` top-8
gives the j-side min (the diagonal lands in the self block where
Y_diag ~ 0 while true neighbors are ~ -300, so host uses top-2 there);
Pool `partition_all_reduce(max)` on blocks c+1..c+3 gives the i-side
min over the tile's 128 j's (the remaining direction comes from other
cores' j-side scans).  This cuts the real-side DVE scans from 96 to 60
j-tiles; the partition reduces ride on the otherwise-idle Pool engine.

Host combines the per-core partials (min over all candidates), gathers
d2 = realNN[argmin], applies sigmoid and the mean.  All reductions are
exact fp32; only the Gram matmuls are bf16.
"""

import numpy as np

N = 12288
D = 256
NCORES = 8
SHARD = N // NCORES          # 1536 rows per core
JTILE = 128                  # j columns per tile (PSUM partitions)
NJT = N // JTILE             # 96 gen j-tiles
RJT = 60                     # real j-tiles: shards c..c+4 (rotated)
FJT = 48                     # j-tiles with a DVE free-side scan (m=0..3)
PAR_LO, PAR_HI = 12, 60      # real j-tiles with partition-reduce harvest
NT = 512                     # i elements per matmul (PSUM bank)
NIT = SHARD // NT            # 3 i-tiles

_cached_nc = None


def _build_nc():
    import concourse.bass_isa as bass_isa
    import concourse.mybir as mybir
    from concourse import bacc
    from concourse.tile import TileContext

    f32 = mybir.dt.float32
    bf16 = mybir.dt.bfloat16
    u32 = mybir.dt.uint32

    nc = bacc.Bacc("TRN2", target_bir_lowering=False, debug=False,
                   num_devices=NCORES)

    colr = nc.dram_tensor("colr", [D, RJT * JTILE], bf16,
                          kind="ExternalInput")
    colg = nc.dram_tensor("colg", [D, N], bf16, kind="ExternalInput")
    auglr = nc.dram_tensor("auglr", [JTILE, RJT * JTILE], bf16,
                           kind="ExternalInput")
    rhs = nc.dram_tensor("rhs", [D, SHARD], bf16, kind="ExternalInput")
    aug = nc.dram_tensor("aug", [4, SHARD], bf16, kind="ExternalInput")
    ones = nc.dram_tensor("ones", [JTILE, JTILE], bf16, kind="ExternalInput")

    o_realv = nc.dram_tensor("o_realv", [128, FJT * 8], f32,
                             kind="ExternalOutput")
    o_par = nc.dram_tensor("o_par", [PAR_HI - PAR_LO, SHARD], f32,
                           kind="ExternalOutput")
    o_genv = nc.dram_tensor("o_genv", [128, NJT * 8], f32,
                            kind="ExternalOutput")
    o_geni = nc.dram_tensor("o_geni", [128, NJT * 8], u32,
                            kind="ExternalOutput")

    with TileContext(nc) as tc:
        with (
            tc.tile_pool(name="const", bufs=1) as constp,
            tc.tile_pool(name="lhs", bufs=4) as lhsp,
            tc.tile_pool(name="wide", bufs=4) as widep,
            tc.tile_pool(name="parp", bufs=2) as parp,
            tc.tile_pool(name="outb", bufs=1) as outp,
            tc.tile_pool(name="ps", bufs=8, space="PSUM") as psp,
        ):
            # Resident rhs: both K-chunks of 2*realT shard, in per-i-tile
            # slices so the first matmul group starts early.
            rhs_sb = constp.tile([128, 2 * SHARD], bf16)
            for it in range(NIT):
                io = it * NT
                nc.sync.dma_start(out=rhs_sb[:, io:io + NT],
                                  in_=rhs[0:128, io:io + NT])
                nc.sync.dma_start(out=rhs_sb[:, SHARD + io:SHARD + io + NT],
                                  in_=rhs[128:256, io:io + NT])
            # aug rhs rows: 0,1 = -hi/lo(|r_i|^2); 2,3 = 1.0; rest zero.
            # (zero-padded to K=128: a K<128 matmul stalls the PE pipeline)
            aug_sb = constp.tile([128, SHARD], bf16)
            nc.vector.memset(aug_sb, 0.0)
            nc.sync.dma_start(out=aug_sb[0:4, :], in_=aug[:, :])
            ones_sb = constp.tile([JTILE, JTILE], bf16)
            nc.sync.dma_start(out=ones_sb[:, :], in_=ones[:, :])

            realv = outp.tile([128, FJT * 8], f32)
            genv = outp.tile([128, NJT * 8], f32)
            geni = outp.tile([128, NJT * 8], u32)

            for jt in range(NJT):
                jo = jt * JTILE
                do_real = jt < RJT
                lhs_g = lhsp.tile([128, 2 * JTILE], bf16, tag="lhs_g")
                nc.sync.dma_start(
                    out=lhs_g[:, :].rearrange("p (c j) -> p c j", c=2),
                    in_=colg[:, jo:jo + JTILE].rearrange(
                        "(c p) j -> p c j", c=2),
                )
                if do_real:
                    lhs_r = lhsp.tile([128, 2 * JTILE], bf16, tag="lhs_r")
                    nc.sync.dma_start(
                        out=lhs_r[:, :].rearrange("p (c j) -> p c j", c=2),
                        in_=colr[:, jo:jo + JTILE].rearrange(
                            "(c p) j -> p c j", c=2),
                    )
                    auglr_t = lhsp.tile([128, JTILE], bf16, tag="auglr_t")
                    nc.sync.dma_start(out=auglr_t[:, :],
                                      in_=auglr[:, jo:jo + JTILE])

                wide_g = widep.tile([128, SHARD], f32, tag="wide_g")
                if do_real:
                    wide_r = widep.tile([128, SHARD], f32, tag="wide_r")

                for it in range(NIT):
                    io = it * NT
                    jobs = [(lhs_g, ones_sb, wide_g)]
                    if do_real:
                        jobs.append((lhs_r, auglr_t, wide_r))
                    for lhs_t, aug_lhs, wide in jobs:
                        ps = psp.tile([128, NT], f32)
                        nc.tensor.matmul(
                            out=ps[:, :],
                            lhsT=lhs_t[:, 0:JTILE],
                            rhs=rhs_sb[:, io:io + NT],
                            start=True, stop=False,
                        )
                        nc.tensor.matmul(
                            out=ps[:, :],
                            lhsT=lhs_t[:, JTILE:2 * JTILE],
                            rhs=rhs_sb[:, SHARD + io:SHARD + io + NT],
                            start=False, stop=False,
                        )
                        nc.tensor.matmul(
                            out=ps[:, :],
                            lhsT=aug_lhs[:, :],
                            rhs=aug_sb[:, io:io + NT],
                            start=False, stop=True,
                        )
                        nc.scalar.activation(
                            out=wide[:, io:io + NT],
                            in_=ps[:, :],
                            func=mybir.ActivationFunctionType.Copy,
                        )

                nc.vector.max(out=genv[:, jt * 8:(jt + 1) * 8],
                              in_=wide_g[:, :])
                nc.vector.max_index(out=geni[:, jt * 8:(jt + 1) * 8],
                                    in_max=genv[:, jt * 8:(jt + 1) * 8],
                                    in_values=wide_g[:, :])
                if do_real:
                    if jt < FJT:
                        nc.vector.max(out=realv[:, jt * 8:(jt + 1) * 8],
                                      in_=wide_r[:, :])
                    if PAR_LO <= jt < PAR_HI:
                        par_t = parp.tile([128, SHARD], f32, tag="par_t")
                        nc.gpsimd.partition_all_reduce(
                            par_t[:, :], wide_r[:, :], channels=128,
                            reduce_op=bass_isa.ReduceOp.max)
                        nc.sync.dma_start(
                            out=o_par[jt - PAR_LO:jt - PAR_LO + 1, :],
                            in_=par_t[0:1, :])

            nc.sync.dma_start(out=o_realv[:, :], in_=realv[:, :])
            nc.sync.dma_start(out=o_genv[:, :], in_=genv[:, :])
            nc.sync.dma_start(out=o_geni[:, :], in_=geni[:, :])

    nc.compile()
    return nc


def _hilo(x, bf):
    hi = x.astype(bf)
    lo = (x - hi.astype(np.float32)).astype(bf)
    return hi, lo


def kernel(real_stats, gen_stats, _trace=False):
    import ml_dtypes
    from concourse.bass_utils import run_bass_kernel_spmd

    bf = ml_dtypes.bfloat16
    global _cached_nc
    real = np.ascontiguousarray(np.asarray(real_stats, dtype=np.float32))
    gen = np.ascontiguousarray(np.asarray(gen_stats, dtype=np.float32))

    realT = np.ascontiguousarray(real.T)                  # [D, N]
    genT = np.ascontiguousarray(gen.T)
    colg_bf = genT.astype(bf)
    rhs_bf = (2.0 * realT).astype(bf)                     # [D, N]
    b2 = np.sum(real.astype(np.float64) ** 2, axis=1).astype(np.float32)
    a2g = np.sum(gen.astype(np.float64) ** 2, axis=1).astype(np.float32)
    ones = np.zeros((JTILE, JTILE), dtype=bf)
    ones[0:2, :] = 1

    RW = RJT * JTILE                                      # 7680 rotated cols
    in_maps = []
    for c in range(NCORES):
        sl = slice(c * SHARD, (c + 1) * SHARD)
        negb2_hi, negb2_lo = _hilo(-b2[sl], bf)
        aug4 = np.zeros((4, SHARD), dtype=bf)
        aug4[0] = negb2_hi
        aug4[1] = negb2_lo
        aug4[2:4] = 1
        colr_rot = np.roll(realT, -c * SHARD, axis=1)[:, :RW]
        a2rot = np.roll(b2, -c * SHARD)[:RW]
        nega2_hi, nega2_lo = _hilo(-a2rot, bf)
        auglr = np.zeros((JTILE, RW), dtype=bf)
        auglr[0:2] = 1
        auglr[2] = nega2_hi
        auglr[3] = nega2_lo
        in_maps.append({
            "colr": colr_rot.astype(bf),
            "colg": colg_bf,
            "auglr": auglr,
            "rhs": np.ascontiguousarray(rhs_bf[:, sl]),
            "aug": aug4,
            "ones": ones,
        })

    if _cached_nc is None:
        _cached_nc = _build_nc()
    res = run_bass_kernel_spmd(_cached_nc, in_maps,
                               core_ids=list(range(NCORES)),
                               trace=_trace)

    # ---- host combine ----
    def grid(name, c, width):
        # [128, width*8] -> [128, width, 8]
        return res.results[c][name].reshape(128, width, 8)

    # real: Y = -dist^2 candidates, min-combined over all sources
    cand = np.full(N, np.inf, dtype=np.float64)
    p_idx = np.arange(128)
    for c in range(NCORES):
        rv = grid("o_realv", c, FJT)                      # [128, FJT, 8] of Y
        top1 = rv[:, :, 0]
        top2 = rv[:, :, 1]
        # self block (k < 12) contains the diagonal: Y_diag ~ 0, true
        # neighbors ~ -300 -> take top2 there when top1 is diag-like
        use2 = np.zeros((128, FJT), dtype=bool)
        use2[:, :12] = top1[:, :12] > -10.0
        y = np.where(use2, top2, top1)                    # [128, FJT]
        jglob = (c * SHARD + np.arange(FJT)[None, :] * JTILE
                 + p_idx[:, None]) % N
        np.minimum.at(cand, jglob.ravel(), (-y).ravel())
        par = res.results[c]["o_par"]                     # [48, SHARD] of Y
        par_min = -par.max(axis=0)                        # min dist^2 per i
        sl = slice(c * SHARD, (c + 1) * SHARD)
        cand[sl] = np.minimum(cand[sl], par_min)
    realNN = np.sqrt(np.maximum(cand, 0.0))               # [N]

    # gen: X = 2G - |r_i|^2;  d1^2 = a2g - max X
    j = np.arange(N)
    genv = np.stack([grid("o_genv", c, NJT)[:, :, 0] for c in range(NCORES)])
    geni = np.stack([res.results[c]["o_geni"].reshape(128, NJT, 8)[:, :, 0]
                     for c in range(NCORES)])
    # [8, 128, NJT] -> [8, N] with j = jt*128 + p
    gv = genv.transpose(0, 2, 1).reshape(NCORES, N)
    gi = geni.transpose(0, 2, 1).reshape(NCORES, N)
    cstar = gv.argmax(axis=0)
    d1 = np.sqrt(np.maximum(a2g - gv[cstar, j], 0.0))
    istar = cstar * SHARD + gi[cstar, j]
    d2 = realNN[istar]

    z = (d2 - d1) / 0.1
    authen = np.where(z >= 0, 1.0 / (1.0 + np.exp(-np.abs(z))),
                      np.exp(-np.abs(z)) / (1.0 + np.exp(-np.abs(z))))
    out = np.asarray(-100.0 * np.mean(authen), dtype=np.float32)
    if _trace:
        return out, res
    return out
